# revision 1
# baseline (speedup 1.0000x reference)
"""Bass/Trainium2 kernel for nn_DenoisedSasrec — fp8 DoubleRow hi/lo version.

Data-parallel over batch: 32 sequences -> 8 NeuronCores x 4 sequences.

Math (per sequence):
    x  = item_emb[positives] + pos_emb                     (L, D)
    z  = silu(x @ Wz^T);  v = silu(x @ Wv^T)
    s  = z @ A @ z^T,  A = Wq^T diag(gq*gk) Wk             (betas zero)
    a  = relu(s * m * w2)^2,   out = a^T @ v

Chain matmuls (z, t=z@A, s) run as 3-term hi/lo e4m3 DoubleRow:
X@W ~= Xh@Wh + Xl@Wh + Xh@Wl where Xh=e4m3(X), Xl=e4m3(X-Xh) (same
scale), giving ~bf16 accuracy at 0.75x bf16 PE-cycles. v-mm (Xh@Wvh)
and out-mm (aq@vq, e5m2) are single fp8 DoubleRow (0.25x).

Scales: xpair = 32*x, Wz pair = 16*Wz^T, psum_z = 512*(x@Wz^T),
z bf16 true, zpair = 128*z, A pair = 2^21*A, psum_t = 2^28*t,
tpair = psum_t/512 = 2^19*t, psum_s = 2^26*s, w2k = 2^5*w2 (bf16),
u = 2^31*(s*m*w2) bf16, aq = relu(u)*u = 2^62*a (e5m2), vq = v (e5m2),
out = psum_o * 2^-62.  Diagonal (mask-exempt) via dwk = (1-m)*2^5*
diag(w2) added on diagonal blocks before relu^2.
"""

import os
import sys

import numpy as np
import ml_dtypes

for _p in ("/opt/trn_rl_repo", "/root/.axon_site/_ro/trn_rl_repo"):
    if os.path.isdir(_p) and _p not in sys.path:
        sys.path.append(_p)

B, L, D = 32, 1024, 1024
N_ITEMS = 50000
NCORES = 8
BPC = B // NCORES
P = 128
NT = L // P
NS = L // 512

S_X = 32.0
S_W = 16.0
C_Z = 128.0
C_A = 2.0**21
S_T = 1.0 / 512
K_W2 = 2.0**5
OUT_DS = 2.0**-62

E4np = ml_dtypes.float8_e4m3
E5np = ml_dtypes.float8_e5m2
BFnp = ml_dtypes.bfloat16

_CACHE = {}


def _build_bass():
    import concourse.bass as bass
    import concourse.bacc as bacc
    import concourse.mybir as mybir
    import concourse.tile as tile
    from concourse.bass import ts, ds
    from concourse.masks import make_identity
    from contextlib import ExitStack

    f32 = mybir.dt.float32
    bf16 = mybir.dt.bfloat16
    e4 = mybir.dt.float8e4
    e5 = mybir.dt.float8e5
    i32 = mybir.dt.int32
    AF = mybir.ActivationFunctionType
    OP = mybir.AluOpType
    DR = mybir.MatmulPerfMode.DoubleRow

    nc = bacc.Bacc("TRN2", target_bir_lowering=False, debug=False, num_devices=NCORES)

    emb_h = nc.declare_dram_parameter("emb", [N_ITEMS + 1, D], bf16, isOutput=False)
    pos_h = nc.declare_dram_parameter("posm", [P, NT, D], bf16, isOutput=False)
    idx_h = nc.declare_dram_parameter("idx", [P, BPC * NT], i32, isOutput=False)
    wzh_h = nc.declare_dram_parameter("wzh", [P, NT, L], e4, isOutput=False)
    wzl_h = nc.declare_dram_parameter("wzl", [P, NT, L], e4, isOutput=False)
    wvh_h = nc.declare_dram_parameter("wvh", [P, NT, D], e4, isOutput=False)
    wvl_h = nc.declare_dram_parameter("wvl", [P, NT, D], e4, isOutput=False)
    amh_h = nc.declare_dram_parameter("amh", [P, NT, L], e4, isOutput=False)
    aml_h = nc.declare_dram_parameter("aml", [P, NT, L], e4, isOutput=False)
    w2_h = nc.declare_dram_parameter("w2k", [P, NT, L], bf16, isOutput=False)
    w2m3_h = nc.declare_dram_parameter("w2km3", [P, NT, L], bf16, isOutput=False)
    mk_h = nc.declare_dram_parameter("maskv", [BPC, P, NT], f32, isOutput=False)
    dw_h = nc.declare_dram_parameter("dwk", [BPC, P, NT], f32, isOutput=False)
    out_h = nc.declare_dram_parameter("out", [BPC * L, D], f32, isOutput=True)

    def mm(ps, lhsT, rhs, start, stop, perf_mode=DR):
        nc.tensor.matmul(out=ps, lhsT=lhsT, rhs=rhs, start=start, stop=stop,
                         perf_mode=perf_mode)

    def mm3(ps, lhs_h, lhs_l, rhs_h, rhs_l, lcol, rcol):
        """3-term paired contraction over NT k-planes into one psum.
        lhs*/rhs* are [P, NT, F] tiles; lcol/rcol slice their free dim."""
        terms = [(lhs_h, rhs_h), (lhs_l, rhs_h), (lhs_h, rhs_l)]
        n = NT // 2
        for t_i, (lt, rt) in enumerate(terms):
            for i in range(n):
                mm(ps, lt[:, 2 * i : 2 * i + 2, lcol], rt[:, 2 * i : 2 * i + 2, rcol],
                   t_i == 0 and i == 0, t_i == 2 and i == n - 1)

    with ExitStack() as ctx:
        tc = ctx.enter_context(tile.TileContext(nc))

        const_p = ctx.enter_context(tc.tile_pool(name="const", bufs=1))
        small_p = ctx.enter_context(tc.tile_pool(name="small", bufs=4))
        xg_p = ctx.enter_context(tc.tile_pool(name="xg", bufs=2))
        big_p = ctx.enter_context(tc.tile_pool(name="big", bufs=1))
        aq_p = ctx.enter_context(tc.tile_pool(name="aq", bufs=1))
        u_p = ctx.enter_context(tc.tile_pool(name="u", bufs=4))
        zs_p = ctx.enter_context(tc.tile_pool(name="zs", bufs=3))
        ob_p = ctx.enter_context(tc.tile_pool(name="ob", bufs=3))
        ps_a = ctx.enter_context(tc.tile_pool(name="ps_a", bufs=6, space="PSUM"))
        ps_t = ctx.enter_context(tc.tile_pool(name="ps_t", bufs=2, space="PSUM"))

        identf = const_p.tile([P, P], f32)
        make_identity(nc, identf[:])
        identb = const_p.tile([P, P], bf16)
        make_identity(nc, identb[:])

        # weight tiles declared up front, loaded lazily (just before first
        # use) so batch-0's gather pipeline isn't queued behind 7MB of
        # weight DMA
        wzh = const_p.tile([P, NT, L], e4)
        wzl = const_p.tile([P, NT, L], e4)
        wvh = const_p.tile([P, NT, D], e4)
        wvl = const_p.tile([P, NT, D], e4)
        amh = const_p.tile([P, NT, L], e4)
        aml = const_p.tile([P, NT, L], e4)
        w2k = const_p.tile([P, NT, L], bf16)

        it_all = const_p.tile([P, BPC * NT], i32)
        nc.gpsimd.dma_start(out=it_all[:], in_=idx_h[:])

        def emit_gathers(b):
            # pos-fill + gather-add interleaved per l_t so transposes can
            # start after the first tile instead of the full 4MB
            xg = xg_p.tile([P, NT, D], bf16, tag="xg", name=f"xg_{b}")
            for l_t in range(NT):
                nc.scalar.dma_start(out=xg[:, l_t, :], in_=pos_h[:, l_t, :])
                nc.gpsimd.indirect_dma_start(
                    out=xg[:, l_t, :],
                    out_offset=None,
                    in_=emb_h[:],
                    in_offset=bass.IndirectOffsetOnAxis(
                        ap=it_all[:, b * NT + l_t : b * NT + l_t + 1], axis=0
                    ),
                    compute_op=OP.add,
                )
            return xg

        xg_next = emit_gathers(0)
        nc.sync.dma_start(out=wzh[:], in_=wzh_h[:])
        nc.sync.dma_start(out=wzl[:], in_=wzl_h[:])

        warm_ps = ps_a.tile([P, 512], f32, space="PSUM", tag="ps", name="warm_ps")
        for wi in range(48):
            nc.tensor.transpose(
                out=warm_ps[:, ts(wi % 4, P)], in_=identf[:], identity=identf[:]
            )

        def emit_transposes(b, xg_cur):
            # transpose emb rows, add posT on-chip, hi/lo quantize:
            # xh/xl[:, d_t, l] = pair(32*(emb^T + pos^T))
            xh = big_p.tile([P, NT, L], e4, tag="xh", name=f"xh_{b}")
            xl = big_p.tile([P, NT, L], e4, tag="xl", name=f"xl_{b}")
            for l_t in range(NT):
                pt = ps_t.tile([P, NT, P], bf16, space="PSUM", tag="pt")
                for d_t in range(NT):
                    nc.tensor.transpose(
                        out=pt[:, d_t, :], in_=xg_cur[:, l_t, ts(d_t, P)],
                        identity=identb[:],
                    )
                nc.scalar.activation(
                    out=xh[:, :, ts(l_t, P)], in_=pt[:], func=AF.Copy, scale=S_X
                )
                nc.vector.scalar_tensor_tensor(
                    out=xl[:, :, ts(l_t, P)], in0=pt[:], scalar=S_X,
                    in1=xh[:, :, ts(l_t, P)], op0=OP.mult, op1=OP.subtract,
                )
            return xh, xl

        xp_next = emit_transposes(0, xg_next)
        for b in range(BPC):
            if b == BPC - 1:
                # reload w2k with the mask+diag folded variant for the last
                # batch: its attention then needs no mask-scalar and no
                # diagonal-fix DVE ops, shrinking the final DVE drain
                nc.sync.dma_start(out=w2k[:], in_=w2m3_h[:])
            maskv = small_p.tile([P, NT], f32, tag="maskv")
            nc.sync.dma_start(out=maskv[:], in_=mk_h[b])
            dwk = small_p.tile([P, NT], f32, tag="dwk")
            nc.sync.dma_start(out=dwk[:], in_=dw_h[b])

            # transposes for batch b were emitted at the tail of the previous
            # batch's attention phase (or in the preamble for b=0)
            xh, xl = xp_next
            xp_next = None

            # ---- z = silu(x @ Wz^T): zh/zl[e, l] = pair(128*z)
            zh = big_p.tile([P, NT, L], e4, tag="zh", name=f"zh_{b}")
            zl = big_p.tile([P, NT, L], e4, tag="zl", name=f"zl_{b}")
            for l_s in range(NS):
                for e_t in range(NT):
                    ps = ps_a.tile([P, 512], f32, space="PSUM", tag="ps")
                    mm3(ps[:], wzh, wzl, xh, xl, ts(e_t, P), ts(l_s, 512))
                    zsl = zs_p.tile([P, 512], bf16, tag="zs")
                    nc.scalar.activation(
                        out=zsl[:], in_=ps[:], func=AF.Silu, scale=1.0 / (S_X * S_W)
                    )
                    nc.scalar.activation(
                        out=zh[:, e_t, ts(l_s, 512)], in_=zsl[:], func=AF.Copy,
                        scale=C_Z,
                    )
                    nc.vector.scalar_tensor_tensor(
                        out=zl[:, e_t, ts(l_s, 512)], in0=zsl[:], scalar=C_Z,
                        in1=zh[:, e_t, ts(l_s, 512)], op0=OP.mult, op1=OP.subtract,
                    )

            # ---- v = silu(x @ Wv^T): vq[j, d] bf16 true scale (3-term);
            # d_s outer so the first 8 psums need only the first wv half
            if b == 0:
                nc.sync.dma_start(out=wvh[:, :, ts(0, 512)],
                                  in_=wvh_h[:, :, ts(0, 512)])
                nc.sync.dma_start(out=wvl[:, :, ts(0, 512)],
                                  in_=wvl_h[:, :, ts(0, 512)])
                nc.sync.dma_start(out=wvh[:, :, ts(1, 512)],
                                  in_=wvh_h[:, :, ts(1, 512)])
                nc.sync.dma_start(out=wvl[:, :, ts(1, 512)],
                                  in_=wvl_h[:, :, ts(1, 512)])
            vq = big_p.tile([P, NT, D], bf16, tag="vq", name=f"vq_{b}")
            for d_s in range(NS):
                for j_t in range(NT):
                    ps = ps_a.tile([P, 512], f32, space="PSUM", tag="ps")
                    mm3(ps[:], xh, xl, wvh, wvl, ts(j_t, P), ts(d_s, 512))
                    nc.scalar.activation(
                        out=vq[:, j_t, ts(d_s, 512)], in_=ps[:], func=AF.Silu,
                        scale=1.0 / (S_X * S_W),
                    )

            # ---- t = z @ A: th/tl[e, i] = pair(psum_t/512) = pair(2^19*t)
            if b == 0:
                for c in range(4):
                    nc.sync.dma_start(out=amh[:, :, ds(c * 256, 256)],
                                      in_=amh_h[:, :, ds(c * 256, 256)])
                    nc.sync.dma_start(out=aml[:, :, ds(c * 256, 256)],
                                      in_=aml_h[:, :, ds(c * 256, 256)])
                nc.sync.dma_start(out=w2k[:], in_=w2_h[:])
            th = big_p.tile([P, NT, L], e4, tag="th", name=f"th_{b}")
            tl = big_p.tile([P, NT, L], e4, tag="tl", name=f"tl_{b}")
            for i_s in range(NS):
                for e_t in range(NT):
                    ps = ps_a.tile([P, 512], f32, space="PSUM", tag="ps")
                    mm3(ps[:], amh, aml, zh, zl, ts(e_t, P), ts(i_s, 512))
                    nc.scalar.activation(
                        out=th[:, e_t, ts(i_s, 512)], in_=ps[:], func=AF.Copy,
                        scale=S_T,
                    )
                    nc.vector.scalar_tensor_tensor(
                        out=tl[:, e_t, ts(i_s, 512)], in0=ps[:], scalar=S_T,
                        in1=th[:, e_t, ts(i_s, 512)], op0=OP.mult, op1=OP.subtract,
                    )

            if b + 1 < BPC:
                xg_next = emit_gathers(b + 1)

            # ---- attention, explicitly scheduled so PE never waits on the
            # DVE->Pool aq chain: lag-2 emissions, next batch's transposes
            # interleaved before the final emission
            aq0 = aq_p.tile([P, 4, 512], bf16, tag="aq0", name=f"aq0_{b}")
            aq1 = aq_p.tile([P, NT, 512], bf16, tag="aq1", name=f"aq1_{b}")

            def s_step(i_s, j_t):
                aq = aq0 if i_s == 0 else aq1
                off = max(j_t * P - i_s * 512, 0)
                w_c = 512 - off
                ao = i_s * 512 + off
                sps = ps_a.tile([P, 512], f32, space="PSUM", tag="ps")
                terms = [(zh, th), (zl, th), (zh, tl)]
                for t_i, (lt, rt) in enumerate(terms):
                    for i in range(NT // 2):
                        mm(sps[:, :w_c], lt[:, 2 * i : 2 * i + 2, ts(j_t, P)],
                           rt[:, 2 * i : 2 * i + 2, ds(ao, w_c)],
                           t_i == 0 and i == 0, t_i == 2 and i == NT // 2 - 1)
                u = u_p.tile([P, 512], bf16, tag="u")
                if b == BPC - 1:
                    nc.vector.tensor_tensor(
                        out=u[:, :w_c], in0=sps[:, :w_c],
                        in1=w2k[:, j_t, ds(ao, w_c)], op=OP.mult,
                    )
                else:
                    nc.vector.scalar_tensor_tensor(
                        out=u[:, :w_c], in0=sps[:, :w_c],
                        scalar=maskv[:, j_t : j_t + 1],
                        in1=w2k[:, j_t, ds(ao, w_c)],
                        op0=OP.mult, op1=OP.mult,
                    )
                dsub = j_t * P - i_s * 512 - off
                if b != BPC - 1 and 0 <= dsub < w_c:
                    dg = u_p.tile([P, P], bf16, tag="dg")
                    nc.vector.scalar_tensor_tensor(
                        out=dg[:], in0=sps[:, ds(dsub, P)],
                        scalar=dwk[:, j_t : j_t + 1], in1=identb[:],
                        op0=OP.mult, op1=OP.mult,
                    )
                    nc.vector.tensor_add(
                        out=u[:, ds(dsub, P)], in0=u[:, ds(dsub, P)], in1=dg[:]
                    )
                # aq = relu(u)*u in one DVE op (keeps the aq chain latency
                # at ~1.4us so the lagged emissions never stall PE)
                nc.vector.scalar_tensor_tensor(
                    out=aq[:, j_t, ds(off, w_c)], in0=u[:, :w_c],
                    scalar=0.0, in1=u[:, :w_c], op0=OP.max, op1=OP.mult,
                )

            def emit_out(ig):
                i_s = 0 if ig < 4 else 1
                aq = aq0 if i_s == 0 else aq1
                i_t = ig - i_s * 4
                for d_s in range(NS):
                    ops = ps_a.tile([P, 512], f32, space="PSUM", tag="ps")
                    for j2 in range(ig + 1):
                        mm(ops[:], aq[:, j2, ts(i_t, P)],
                           vq[:, j2, ts(d_s, 512)],
                           j2 == 0, j2 == ig, perf_mode=None)
                    ot = ob_p.tile([P, 512], f32, tag="ob")
                    nc.scalar.activation(
                        out=ot[:], in_=ops[:], func=AF.Copy, scale=OUT_DS
                    )
                    nc.sync.dma_start(
                        out=out_h[b * L + ig * P : b * L + (ig + 1) * P,
                                  ts(d_s, 512)],
                        in_=ot[:],
                    )

            s_step(0, 0); s_step(0, 1); s_step(0, 2); s_step(0, 3)
            s_step(1, 0); emit_out(0)
            s_step(1, 1); emit_out(1)
            s_step(1, 2); emit_out(2)
            s_step(1, 3); emit_out(3)
            s_step(1, 4); s_step(1, 5); s_step(1, 6); s_step(1, 7)
            emit_out(4); emit_out(5)
            if b + 1 < BPC:
                xp_next = emit_transposes(b + 1, xg_next)
            emit_out(6); emit_out(7)

    nc.compile()
    return nc


def _host_prep(positives, mask, item_emb, pos_emb, Wz, Wv, Wq, Wk,
               gamma_q, beta_q, gamma_k, beta_k, sparse_w):
    gq, gk = np.asarray(gamma_q[0]), np.asarray(gamma_k[0])
    bq, bk = np.asarray(beta_q[0]), np.asarray(beta_k[0])
    assert np.abs(bq).max() == 0.0 and np.abs(bk).max() == 0.0
    Wq = np.asarray(Wq, np.float32)
    Wk = np.asarray(Wk, np.float32)
    sw = np.asarray(sparse_w, np.float32)
    scale = 1.0 / np.float32(np.sqrt(float(L) * float(D)))

    amat = ((Wq.T * (gq * gk)[None, :].astype(np.float32)) @ Wk).astype(np.float32)
    w2T = (np.triu(np.ones((L, L), np.float32)) * sw.T * scale).astype(np.float32)
    mk = np.asarray(mask, np.float32)
    dwkv = ((1.0 - mk) * np.diag(sw)[None] * scale * K_W2).astype(np.float32)

    def tiled(a, dt):
        return np.ascontiguousarray(
            np.asarray(a).reshape(NT, P, -1).transpose(1, 0, 2).astype(dt)
        )

    def pair(a):
        hi = np.asarray(a, np.float32).astype(E4np).astype(np.float32)
        lo = (a - hi).astype(E4np)
        return hi.astype(E4np), lo

    wzp = pair(np.asarray(Wz, np.float32).T * S_W)
    wvp = pair(np.asarray(Wv, np.float32).T * S_W)
    amp = pair(amat * C_A)

    idx = np.ascontiguousarray(
        np.asarray(positives).astype(np.int32)
        .reshape(NCORES, BPC, NT, P).transpose(0, 3, 1, 2)
        .reshape(NCORES, P, BPC * NT)
    )
    maskv = np.ascontiguousarray(mk.reshape(B, NT, P).transpose(0, 2, 1))
    dwv = np.ascontiguousarray(dwkv.reshape(B, NT, P).transpose(0, 2, 1))

    # folded w2 for each core's last batch: w2*m_j off-diag, w2_jj on diag
    mk3 = mk[BPC - 1::BPC]                                   # (NCORES, L)
    w2km3 = (w2T[None] * K_W2 * mk3[:, :, None]
             + (np.eye(L, dtype=np.float32) * np.diag(w2T)[None, :] * K_W2)[None]
             * (1.0 - mk3)[:, :, None]).astype(BFnp)         # (NCORES, L, L)

    common = {
        "emb": np.ascontiguousarray(np.asarray(item_emb, np.float32).astype(BFnp)),
        "posm": tiled(np.asarray(pos_emb, np.float32), BFnp),
        "wzh": tiled(wzp[0], E4np),
        "wzl": tiled(wzp[1], E4np),
        "wvh": tiled(wvp[0], E4np),
        "wvl": tiled(wvp[1], E4np),
        "amh": tiled(amp[0], E4np),
        "aml": tiled(amp[1], E4np),
        "w2k": tiled(w2T * K_W2, BFnp),
    }
    in_maps = []
    for c in range(NCORES):
        sl = slice(c * BPC, (c + 1) * BPC)
        m = dict(common)
        m["idx"] = np.ascontiguousarray(idx[c])
        m["w2km3"] = np.ascontiguousarray(
            w2km3[c].reshape(NT, P, L).transpose(1, 0, 2)
        )
        m["maskv"] = np.ascontiguousarray(maskv[sl])
        m["dwk"] = np.ascontiguousarray(dwv[sl])
        in_maps.append(m)
    return in_maps


def _get_nc():
    if "nc" not in _CACHE:
        _CACHE["nc"] = _build_bass()
    return _CACHE["nc"]


def kernel(**inputs) -> np.ndarray:
    from concourse.bass_utils import run_bass_kernel_spmd

    in_maps = _host_prep(**inputs)
    nc = _get_nc()
    res = run_bass_kernel_spmd(
        nc, in_maps, core_ids=list(range(NCORES)),
        **_CACHE.get("run_kwargs", {}),
    )
    out = np.concatenate(
        [r["out"].reshape(BPC, L, D) for r in res.results], axis=0
    )
    _CACHE["last_results"] = res
    return out


if __name__ == "__main__":
    nc = _get_nc()
    print("built bass module OK")



# revision 27
# speedup vs baseline: 1.0364x; 1.0364x over previous
"""Bass/Trainium2 kernel for nn_DenoisedSasrec — fp8 DoubleRow hi/lo version.

Data-parallel over batch: 32 sequences -> 8 NeuronCores x 4 sequences.

Math (per sequence):
    x  = item_emb[positives] + pos_emb                     (L, D)
    z  = silu(x @ Wz^T);  v = silu(x @ Wv^T)
    s  = z @ A @ z^T,  A = Wq^T diag(gq*gk) Wk             (betas zero)
    a  = relu(s * m * w2)^2,   out = a^T @ v

Chain matmuls (z, t=z@A, s) run as 3-term hi/lo e4m3 DoubleRow:
X@W ~= Xh@Wh + Xl@Wh + Xh@Wl where Xh=e4m3(X), Xl=e4m3(X-Xh) (same
scale), giving ~bf16 accuracy at 0.75x bf16 PE-cycles. The v-mm drops
half the Xl-term volume (Xl contracts only d-planes 0-3) — the
x-residual noise it adds to v stays within the rel-err budget. The
out-mm (aq@vq) is bf16; the u/zsl intermediates stay f32 (same engine
cost) to preserve budget.

Scales: xpair = 32*x, Wz pair = 16*Wz^T, psum_z = 512*(x@Wz^T),
z bf16 true, zpair = 128*z, A pair = 2^21*A, psum_t = 2^28*t,
tpair = psum_t/512 = 2^19*t, psum_s = 2^26*s, w2k = 2^5*w2 (bf16),
u = 2^31*(s*m*w2) bf16, aq = relu(u)*u = 2^62*a (bf16), vq = v (bf16),
out = psum_o * 2^-62.  Diagonal (mask-exempt) via dwk = (1-m)*2^5*
diag(w2) added on diagonal blocks before relu^2.
"""

import os
import sys

import numpy as np
import ml_dtypes

for _p in ("/opt/trn_rl_repo", "/root/.axon_site/_ro/trn_rl_repo"):
    if os.path.isdir(_p) and _p not in sys.path:
        sys.path.append(_p)

B, L, D = 32, 1024, 1024
N_ITEMS = 50000
NCORES = 8
BPC = B // NCORES
P = 128
NT = L // P
NS = L // 512

S_X = 32.0
S_W = 16.0
C_Z = 128.0
C_A = 2.0**21
S_T = 1.0 / 512
K_W2 = 2.0**5
OUT_DS = 2.0**-62

E4np = ml_dtypes.float8_e4m3
E5np = ml_dtypes.float8_e5m2
BFnp = ml_dtypes.bfloat16

_CACHE = {}


def _build_bass():
    import concourse.bass as bass
    import concourse.bacc as bacc
    import concourse.mybir as mybir
    import concourse.tile as tile
    from concourse.bass import ts, ds
    from concourse.masks import make_identity
    from contextlib import ExitStack

    f32 = mybir.dt.float32
    bf16 = mybir.dt.bfloat16
    e4 = mybir.dt.float8e4
    e5 = mybir.dt.float8e5
    i32 = mybir.dt.int32
    AF = mybir.ActivationFunctionType
    OP = mybir.AluOpType
    DR = mybir.MatmulPerfMode.DoubleRow

    nc = bacc.Bacc("TRN2", target_bir_lowering=False, debug=False, num_devices=NCORES)

    emb_h = nc.declare_dram_parameter("emb", [N_ITEMS + 1, D], bf16, isOutput=False)
    pos_h = nc.declare_dram_parameter("posm", [P, NT, D], bf16, isOutput=False)
    idx_h = nc.declare_dram_parameter("idx", [P, BPC * NT], i32, isOutput=False)
    wzh_h = nc.declare_dram_parameter("wzh", [P, NT, L], e4, isOutput=False)
    wzl_h = nc.declare_dram_parameter("wzl", [P, NT, L], e4, isOutput=False)
    wvh_h = nc.declare_dram_parameter("wvh", [P, NT, D], e4, isOutput=False)
    wvl_h = nc.declare_dram_parameter("wvl", [P, NT, D], e4, isOutput=False)
    amh_h = nc.declare_dram_parameter("amh", [P, NT, L], e4, isOutput=False)
    aml_h = nc.declare_dram_parameter("aml", [P, NT, L], e4, isOutput=False)
    w2_h = nc.declare_dram_parameter("w2k", [P, NT, L], bf16, isOutput=False)
    w2m3_h = nc.declare_dram_parameter("w2km3", [P, NT, L], bf16, isOutput=False)
    mk_h = nc.declare_dram_parameter("maskv", [BPC, P, NT], f32, isOutput=False)
    dw_h = nc.declare_dram_parameter("dwk", [BPC, P, NT], f32, isOutput=False)
    out_h = nc.declare_dram_parameter("out", [BPC * L, D], f32, isOutput=True)

    def mm(ps, lhsT, rhs, start, stop, perf_mode=DR):
        nc.tensor.matmul(out=ps, lhsT=lhsT, rhs=rhs, start=start, stop=stop,
                         perf_mode=perf_mode)

    def mm3(ps, lhs_h, lhs_l, rhs_h, rhs_l, lcol, rcol):
        """3-term paired contraction over NT k-planes into one psum.
        lhs*/rhs* are [P, NT, F] tiles; lcol/rcol slice their free dim."""
        terms = [(lhs_h, rhs_h), (lhs_l, rhs_h), (lhs_h, rhs_l)]
        n = NT // 2
        for t_i, (lt, rt) in enumerate(terms):
            for i in range(n):
                mm(ps, lt[:, 2 * i : 2 * i + 2, lcol], rt[:, 2 * i : 2 * i + 2, rcol],
                   t_i == 0 and i == 0, t_i == 2 and i == n - 1)

    def mm25(ps, lhs_h, lhs_l, rhs_h, rhs_l, lcol, rcol):
        """2.5-term contraction: lhs_h@rhs_h + lhs_l@rhs_h on the first
        half of the k-planes only + lhs_h@rhs_l. Drops half the lhs-lo
        volume; the residual noise stays within the rel-err budget."""
        n = NT // 2
        for i in range(n):
            mm(ps, lhs_h[:, 2 * i : 2 * i + 2, lcol],
               rhs_h[:, 2 * i : 2 * i + 2, rcol], i == 0, False)
        for i in range(n // 2):
            mm(ps, lhs_l[:, 2 * i : 2 * i + 2, lcol],
               rhs_h[:, 2 * i : 2 * i + 2, rcol], False, False)
        for i in range(n):
            mm(ps, lhs_h[:, 2 * i : 2 * i + 2, lcol],
               rhs_l[:, 2 * i : 2 * i + 2, rcol], False, i == n - 1)

    with ExitStack() as ctx:
        tc = ctx.enter_context(tile.TileContext(nc))

        const_p = ctx.enter_context(tc.tile_pool(name="const", bufs=1))
        small_p = ctx.enter_context(tc.tile_pool(name="small", bufs=4))
        xg_p = ctx.enter_context(tc.tile_pool(name="xg", bufs=2))
        big_p = ctx.enter_context(tc.tile_pool(name="big", bufs=1))
        aq_p = ctx.enter_context(tc.tile_pool(name="aq", bufs=1))
        u_p = ctx.enter_context(tc.tile_pool(name="u", bufs=4))
        zs_p = ctx.enter_context(tc.tile_pool(name="zs", bufs=3))
        ob_p = ctx.enter_context(tc.tile_pool(name="ob", bufs=3))
        ps_a = ctx.enter_context(tc.tile_pool(name="ps_a", bufs=6, space="PSUM"))
        ps_t = ctx.enter_context(tc.tile_pool(name="ps_t", bufs=2, space="PSUM"))

        identf = const_p.tile([P, P], f32)
        make_identity(nc, identf[:])
        identb = const_p.tile([P, P], bf16)
        make_identity(nc, identb[:])

        # weight tiles declared up front, loaded lazily (just before first
        # use) so batch-0's gather pipeline isn't queued behind 7MB of
        # weight DMA
        wzh = const_p.tile([P, NT, L], e4)
        wzl = const_p.tile([P, NT, L], e4)
        wvh = const_p.tile([P, NT, D], e4)
        wvl = const_p.tile([P, NT, D], e4)
        amh = const_p.tile([P, NT, L], e4)
        aml = const_p.tile([P, NT, L], e4)
        w2k = const_p.tile([P, NT, L], bf16)

        it_all = const_p.tile([P, BPC * NT], i32)
        nc.gpsimd.dma_start(out=it_all[:], in_=idx_h[:])

        def emit_gathers(b):
            # pos-fill + gather-add interleaved per l_t so transposes can
            # start after the first tile instead of the full 4MB
            xg = xg_p.tile([P, NT, D], bf16, tag="xg", name=f"xg_{b}")
            for l_t in range(NT):
                nc.scalar.dma_start(out=xg[:, l_t, :], in_=pos_h[:, l_t, :])
                nc.gpsimd.indirect_dma_start(
                    out=xg[:, l_t, :],
                    out_offset=None,
                    in_=emb_h[:],
                    in_offset=bass.IndirectOffsetOnAxis(
                        ap=it_all[:, b * NT + l_t : b * NT + l_t + 1], axis=0
                    ),
                    compute_op=OP.add,
                )
                if b == 0 and l_t == 1:
                    # release the wzl load only after gather 1 (data-dep
                    # Pool copy) so its 2.9us transfer can't sit ahead of
                    # the batch-0 gather stream; wzh loads up front in the
                    # otherwise-idle early window
                    nc.gpsimd.tensor_copy(out=wzl[0:1, 0, 0:1],
                                          in_=xg[0:1, l_t, 0:1])
                    nc.sync.dma_start(out=wzl[:], in_=wzl_h[:])
            return xg

        xg_next = emit_gathers(0)
        nc.sync.dma_start(out=wzh[:], in_=wzh_h[:])

        warm_ps = ps_a.tile([P, 512], f32, space="PSUM", tag="ps", name="warm_ps")
        for wi in range(48):
            nc.tensor.transpose(
                out=warm_ps[:, ts(wi % 4, P)], in_=identf[:], identity=identf[:]
            )

        def emit_transposes(b, xg_cur):
            # transpose emb rows, add posT on-chip, hi/lo quantize:
            # xh/xl[:, d_t, l] = pair(32*(emb^T + pos^T))
            xh = big_p.tile([P, NT, L], e4, tag="xh", name=f"xh_{b}")
            xl = big_p.tile([P, NT, L], e4, tag="xl", name=f"xl_{b}")
            for l_t in range(NT):
                pt = ps_t.tile([P, NT, P], bf16, space="PSUM", tag="pt")
                for d_t in range(NT):
                    nc.tensor.transpose(
                        out=pt[:, d_t, :], in_=xg_cur[:, l_t, ts(d_t, P)],
                        identity=identb[:],
                    )
                nc.scalar.activation(
                    out=xh[:, :, ts(l_t, P)], in_=pt[:], func=AF.Copy, scale=S_X
                )
                nc.vector.scalar_tensor_tensor(
                    out=xl[:, :, ts(l_t, P)], in0=pt[:], scalar=S_X,
                    in1=xh[:, :, ts(l_t, P)], op0=OP.mult, op1=OP.subtract,
                )
            return xh, xl

        xp_next = emit_transposes(0, xg_next)
        for b in range(BPC):
            if b == BPC - 1:
                # reload w2k with the mask+diag folded variant for the last
                # batch: its attention then needs no mask-scalar and no
                # diagonal-fix DVE ops, shrinking the final DVE drain
                nc.sync.dma_start(out=w2k[:], in_=w2m3_h[:])
            maskv = small_p.tile([P, NT], f32, tag="maskv")
            nc.sync.dma_start(out=maskv[:], in_=mk_h[b])
            dwk = small_p.tile([P, NT], f32, tag="dwk")
            nc.sync.dma_start(out=dwk[:], in_=dw_h[b])

            # transposes for batch b were emitted at the tail of the previous
            # batch's attention phase (or in the preamble for b=0)
            xh, xl = xp_next
            xp_next = None

            # ---- z = silu(x @ Wz^T): zh/zl[e, l] = pair(128*z)
            zh = big_p.tile([P, NT, L], e4, tag="zh", name=f"zh_{b}")
            zl = big_p.tile([P, NT, L], e4, tag="zl", name=f"zl_{b}")
            for l_s in range(NS):
                for e_t in range(NT):
                    ps = ps_a.tile([P, 512], f32, space="PSUM", tag="ps")
                    mm3(ps[:], wzh, wzl, xh, xl, ts(e_t, P), ts(l_s, 512))
                    zsl = zs_p.tile([P, 512], f32, tag="zs")
                    nc.scalar.activation(
                        out=zsl[:], in_=ps[:], func=AF.Silu, scale=1.0 / (S_X * S_W)
                    )
                    nc.scalar.activation(
                        out=zh[:, e_t, ts(l_s, 512)], in_=zsl[:], func=AF.Copy,
                        scale=C_Z,
                    )
                    nc.vector.scalar_tensor_tensor(
                        out=zl[:, e_t, ts(l_s, 512)], in0=zsl[:], scalar=C_Z,
                        in1=zh[:, e_t, ts(l_s, 512)], op0=OP.mult, op1=OP.subtract,
                    )

            # ---- v = silu(x @ Wv^T): vq[j, d] bf16 true scale (3-term);
            # d_s outer so the first 8 psums need only the first wv half
            if b == 0:
                nc.sync.dma_start(out=wvh[:, :, ts(0, 512)],
                                  in_=wvh_h[:, :, ts(0, 512)])
                nc.sync.dma_start(out=wvl[:, :, ts(0, 512)],
                                  in_=wvl_h[:, :, ts(0, 512)])
                nc.sync.dma_start(out=wvh[:, :, ts(1, 512)],
                                  in_=wvh_h[:, :, ts(1, 512)])
                nc.sync.dma_start(out=wvl[:, :, ts(1, 512)],
                                  in_=wvl_h[:, :, ts(1, 512)])
            vq = big_p.tile([P, NT, D], bf16, tag="vq", name=f"vq_{b}")
            for d_s in range(NS):
                for j_t in range(NT):
                    ps = ps_a.tile([P, 512], f32, space="PSUM", tag="ps")
                    mm25(ps[:], xh, xl, wvh, wvl, ts(j_t, P), ts(d_s, 512))
                    nc.scalar.activation(
                        out=vq[:, j_t, ts(d_s, 512)], in_=ps[:], func=AF.Silu,
                        scale=1.0 / (S_X * S_W),
                    )

            # ---- t = z @ A: th/tl[e, i] = pair(psum_t/512) = pair(2^19*t)
            if b == 0:
                for c in range(4):
                    nc.sync.dma_start(out=amh[:, :, ds(c * 256, 256)],
                                      in_=amh_h[:, :, ds(c * 256, 256)])
                    nc.sync.dma_start(out=aml[:, :, ds(c * 256, 256)],
                                      in_=aml_h[:, :, ds(c * 256, 256)])
                nc.sync.dma_start(out=w2k[:], in_=w2_h[:])
            th = big_p.tile([P, NT, L], e4, tag="th", name=f"th_{b}")
            tl = big_p.tile([P, NT, L], e4, tag="tl", name=f"tl_{b}")
            for i_s in range(NS):
                for e_t in range(NT):
                    ps = ps_a.tile([P, 512], f32, space="PSUM", tag="ps")
                    mm3(ps[:], amh, aml, zh, zl, ts(e_t, P), ts(i_s, 512))
                    nc.scalar.activation(
                        out=th[:, e_t, ts(i_s, 512)], in_=ps[:], func=AF.Copy,
                        scale=S_T,
                    )
                    nc.vector.scalar_tensor_tensor(
                        out=tl[:, e_t, ts(i_s, 512)], in0=ps[:], scalar=S_T,
                        in1=th[:, e_t, ts(i_s, 512)], op0=OP.mult, op1=OP.subtract,
                    )

            if b + 1 < BPC:
                xg_next = emit_gathers(b + 1)

            # ---- attention, explicitly scheduled so PE never waits on the
            # DVE->Pool aq chain: lag-2 emissions, next batch's transposes
            # interleaved before the final emission
            aq0 = aq_p.tile([P, 4, 512], bf16, tag="aq0", name=f"aq0_{b}")
            aq1 = aq_p.tile([P, NT, 512], bf16, tag="aq1", name=f"aq1_{b}")

            def s_step(i_s, j_t):
                aq = aq0 if i_s == 0 else aq1
                off = max(j_t * P - i_s * 512, 0)
                w_c = 512 - off
                ao = i_s * 512 + off
                sps = ps_a.tile([P, 512], f32, space="PSUM", tag="ps")
                terms = [(zh, th), (zl, th), (zh, tl)]
                for t_i, (lt, rt) in enumerate(terms):
                    for i in range(NT // 2):
                        mm(sps[:, :w_c], lt[:, 2 * i : 2 * i + 2, ts(j_t, P)],
                           rt[:, 2 * i : 2 * i + 2, ds(ao, w_c)],
                           t_i == 0 and i == 0, t_i == 2 and i == NT // 2 - 1)
                u = u_p.tile([P, 512], f32, tag="u")
                if b == BPC - 1:
                    nc.vector.tensor_tensor(
                        out=u[:, :w_c], in0=sps[:, :w_c],
                        in1=w2k[:, j_t, ds(ao, w_c)], op=OP.mult,
                    )
                else:
                    nc.vector.scalar_tensor_tensor(
                        out=u[:, :w_c], in0=sps[:, :w_c],
                        scalar=maskv[:, j_t : j_t + 1],
                        in1=w2k[:, j_t, ds(ao, w_c)],
                        op0=OP.mult, op1=OP.mult,
                    )
                dsub = j_t * P - i_s * 512 - off
                if b != BPC - 1 and 0 <= dsub < w_c:
                    dg = u_p.tile([P, P], f32, tag="dg")
                    nc.vector.scalar_tensor_tensor(
                        out=dg[:], in0=sps[:, ds(dsub, P)],
                        scalar=dwk[:, j_t : j_t + 1], in1=identb[:],
                        op0=OP.mult, op1=OP.mult,
                    )
                    nc.vector.tensor_add(
                        out=u[:, ds(dsub, P)], in0=u[:, ds(dsub, P)], in1=dg[:]
                    )
                # aq = relu(u)*u in one DVE op (keeps the aq chain latency
                # at ~1.4us so the lagged emissions never stall PE)
                nc.vector.scalar_tensor_tensor(
                    out=aq[:, j_t, ds(off, w_c)], in0=u[:, :w_c],
                    scalar=0.0, in1=u[:, :w_c], op0=OP.max, op1=OP.mult,
                )

            def emit_out(ig):
                i_s = 0 if ig < 4 else 1
                aq = aq0 if i_s == 0 else aq1
                i_t = ig - i_s * 4
                for d_s in range(NS):
                    ops = ps_a.tile([P, 512], f32, space="PSUM", tag="ps")
                    for j2 in range(ig + 1):
                        mm(ops[:], aq[:, j2, ts(i_t, P)],
                           vq[:, j2, ts(d_s, 512)],
                           j2 == 0, j2 == ig, perf_mode=None)
                    ot = ob_p.tile([P, 512], f32, tag="ob")
                    nc.scalar.activation(
                        out=ot[:], in_=ops[:], func=AF.Copy, scale=OUT_DS
                    )
                    nc.sync.dma_start(
                        out=out_h[b * L + ig * P : b * L + (ig + 1) * P,
                                  ts(d_s, 512)],
                        in_=ot[:],
                    )

            s_step(0, 0); s_step(0, 1); s_step(0, 2); s_step(0, 3)
            s_step(1, 0); emit_out(0)
            s_step(1, 1); emit_out(1)
            s_step(1, 2); emit_out(2)
            s_step(1, 3); emit_out(3)
            s_step(1, 4); s_step(1, 5); s_step(1, 6); s_step(1, 7)
            emit_out(4); emit_out(5)
            if b + 1 < BPC:
                xp_next = emit_transposes(b + 1, xg_next)
            emit_out(6); emit_out(7)

    nc.compile()
    return nc


def _host_prep(positives, mask, item_emb, pos_emb, Wz, Wv, Wq, Wk,
               gamma_q, beta_q, gamma_k, beta_k, sparse_w):
    gq, gk = np.asarray(gamma_q[0]), np.asarray(gamma_k[0])
    bq, bk = np.asarray(beta_q[0]), np.asarray(beta_k[0])
    assert np.abs(bq).max() == 0.0 and np.abs(bk).max() == 0.0
    Wq = np.asarray(Wq, np.float32)
    Wk = np.asarray(Wk, np.float32)
    sw = np.asarray(sparse_w, np.float32)
    scale = 1.0 / np.float32(np.sqrt(float(L) * float(D)))

    amat = ((Wq.T * (gq * gk)[None, :].astype(np.float32)) @ Wk).astype(np.float32)
    w2T = (np.triu(np.ones((L, L), np.float32)) * sw.T * scale).astype(np.float32)
    mk = np.asarray(mask, np.float32)
    dwkv = ((1.0 - mk) * np.diag(sw)[None] * scale * K_W2).astype(np.float32)

    def tiled(a, dt):
        return np.ascontiguousarray(
            np.asarray(a).reshape(NT, P, -1).transpose(1, 0, 2).astype(dt)
        )

    def pair(a):
        hi = np.asarray(a, np.float32).astype(E4np).astype(np.float32)
        lo = (a - hi).astype(E4np)
        return hi.astype(E4np), lo

    wzp = pair(np.asarray(Wz, np.float32).T * S_W)
    wvp = pair(np.asarray(Wv, np.float32).T * S_W)
    amp = pair(amat * C_A)

    idx = np.ascontiguousarray(
        np.asarray(positives).astype(np.int32)
        .reshape(NCORES, BPC, NT, P).transpose(0, 3, 1, 2)
        .reshape(NCORES, P, BPC * NT)
    )
    maskv = np.ascontiguousarray(mk.reshape(B, NT, P).transpose(0, 2, 1))
    dwv = np.ascontiguousarray(dwkv.reshape(B, NT, P).transpose(0, 2, 1))

    # folded w2 for each core's last batch: w2*m_j off-diag, w2_jj on diag
    mk3 = mk[BPC - 1::BPC]                                   # (NCORES, L)
    w2km3 = (w2T[None] * K_W2 * mk3[:, :, None]
             + (np.eye(L, dtype=np.float32) * np.diag(w2T)[None, :] * K_W2)[None]
             * (1.0 - mk3)[:, :, None]).astype(BFnp)         # (NCORES, L, L)

    common = {
        "emb": np.ascontiguousarray(np.asarray(item_emb, np.float32).astype(BFnp)),
        "posm": tiled(np.asarray(pos_emb, np.float32), BFnp),
        "wzh": tiled(wzp[0], E4np),
        "wzl": tiled(wzp[1], E4np),
        "wvh": tiled(wvp[0], E4np),
        "wvl": tiled(wvp[1], E4np),
        "amh": tiled(amp[0], E4np),
        "aml": tiled(amp[1], E4np),
        "w2k": tiled(w2T * K_W2, BFnp),
    }
    in_maps = []
    for c in range(NCORES):
        sl = slice(c * BPC, (c + 1) * BPC)
        m = dict(common)
        m["idx"] = np.ascontiguousarray(idx[c])
        m["w2km3"] = np.ascontiguousarray(
            w2km3[c].reshape(NT, P, L).transpose(1, 0, 2)
        )
        m["maskv"] = np.ascontiguousarray(maskv[sl])
        m["dwk"] = np.ascontiguousarray(dwv[sl])
        in_maps.append(m)
    return in_maps


def _get_nc():
    if "nc" not in _CACHE:
        _CACHE["nc"] = _build_bass()
    return _CACHE["nc"]


def kernel(**inputs) -> np.ndarray:
    from concourse.bass_utils import run_bass_kernel_spmd

    in_maps = _host_prep(**inputs)
    nc = _get_nc()
    res = run_bass_kernel_spmd(
        nc, in_maps, core_ids=list(range(NCORES)),
        **_CACHE.get("run_kwargs", {}),
    )
    out = np.concatenate(
        [r["out"].reshape(BPC, L, D) for r in res.results], axis=0
    )
    _CACHE["last_results"] = res
    return out


if __name__ == "__main__":
    nc = _get_nc()
    print("built bass module OK")
